# revision 4
# baseline (speedup 1.0000x reference)
"""Trainium2 Bass kernel for an 8-batch AttentionBlock (GroupNorm + single-head
self-attention over 64x64 spatial + residual), data-parallel over batch on 8
NeuronCores (one batch element per core).

Per-core math (x: [512, 4096] f32):
  h   = groupnorm(x) * gamma + beta                  (32 groups of 16 ch)
  q   = (Wq h + bq) / sqrt(512)   k = Wk h + bk      (bf16, layout [c, i])
  vT  = (Wv h)^T                                     (bf16, layout [j, c])
  St  = k^T q          [j, i] blocks, PSUM f32
  Pt  = exp(St)        bf16 SBUF
  den = ones^T Pt      [1, i]  (PE column-sum accumulation)
  O   = vT^T Pt        [c, i]  PSUM accumulation over j-blocks
  attn= O * (1/den)    bf16 (broadcast multiply)
  out = x + Wp attn + (Wp bv + bp)                   (v-bias folded on host)
"""

import sys

if "/opt/trn_rl_repo" not in sys.path:
    sys.path.insert(0, "/opt/trn_rl_repo")

import math

import ml_dtypes
import numpy as np

C = 512
N = 4096
P = 128
CT = C // P      # 4 channel tiles
FB = 512         # free-dim block (i)
NB = N // FB     # 8 i-blocks
JB = N // P      # 32 j-blocks
GS = 16          # channels per group
EPS = 1e-5
PIPE = 3         # jb-delay between St/exp emission and den/O consumption

_CACHE = {}


def _build():
    import concourse.tile as tile
    from concourse import bacc, mybir

    f32 = mybir.dt.float32
    bf16 = mybir.dt.bfloat16
    AF = mybir.ActivationFunctionType

    nc = bacc.Bacc("TRN2", target_bir_lowering=False, debug=False, num_devices=8)

    x_d = nc.dram_tensor("x", [C, N], f32, kind="ExternalInput").ap()
    wq_d = nc.dram_tensor("wqT", [C, C], bf16, kind="ExternalInput").ap()
    wk_d = nc.dram_tensor("wkT", [C, C], bf16, kind="ExternalInput").ap()
    wv_d = nc.dram_tensor("wvT", [C, C], bf16, kind="ExternalInput").ap()
    wp_d = nc.dram_tensor("wpT", [C, C], bf16, kind="ExternalInput").ap()
    bq_d = nc.dram_tensor("bq", [P, CT], f32, kind="ExternalInput").ap()
    bk_d = nc.dram_tensor("bk", [P, CT], f32, kind="ExternalInput").ap()
    bp_d = nc.dram_tensor("bp", [P, CT], f32, kind="ExternalInput").ap()
    gamma_d = nc.dram_tensor("gamma", [P, CT], f32, kind="ExternalInput").ap()
    beta_d = nc.dram_tensor("beta", [P, CT], f32, kind="ExternalInput").ap()
    g16_d = nc.dram_tensor("g16", [P, P // GS], f32, kind="ExternalInput").ap()
    gt_d = nc.dram_tensor("gt", [P // GS, P], f32, kind="ExternalInput").ap()
    out_d = nc.dram_tensor("out", [C, N], f32, kind="ExternalOutput").ap()

    with tile.TileContext(nc) as tc:
        from contextlib import ExitStack

        with ExitStack() as ctx:
            consts = ctx.enter_context(tc.tile_pool(name="consts", bufs=1))
            big = ctx.enter_context(tc.tile_pool(name="big", bufs=1))

            def load_w(dram, nm):
                t = consts.tile([P, CT, C], bf16, name=nm)
                nc.sync.dma_start(t[:], dram.rearrange("(kt p) m -> p kt m", p=P))
                return t

            wq_sb = load_w(wq_d, "wq_sb")
            wk_sb = load_w(wk_d, "wk_sb")
            wv_sb = load_w(wv_d, "wv_sb")
            wp_sb = load_w(wp_d, "wp_sb")

            def load_small(dram, shape, nm):
                t = consts.tile(shape, f32, name=nm)
                nc.sync.dma_start(t[:], dram)
                return t

            bq_sb = load_small(bq_d, [P, CT], "bq_sb")
            bk_sb = load_small(bk_d, [P, CT], "bk_sb")
            bp_sb = load_small(bp_d, [P, CT], "bp_sb")
            gamma_sb = load_small(gamma_d, [P, CT], "gamma_sb")
            beta_sb = load_small(beta_d, [P, CT], "beta_sb")
            g16_sb = load_small(g16_d, [P, P // GS], "g16_sb")
            gt_sb = load_small(gt_d, [P // GS, P], "gt_sb")

            ones_sb = consts.tile([P, 1], bf16)
            nc.vector.memset(ones_sb[:], 1.0)
            eps_sb = consts.tile([P // GS, 1], f32)
            nc.vector.memset(eps_sb[:], EPS)

            q_sb = big.tile([P, CT, N], bf16, name="q")
            k_sb = big.tile([P, CT, N], bf16, name="k")
            vt_sb = big.tile([P, JB, C], bf16, name="vt")

            # ---------------- phase 1+2: groupnorm -> h -> q/k/vT ----------
            with tc.tile_pool(name="hpool", bufs=1) as hp:
                h_sb = hp.tile([P, CT, N], bf16, name="h")
                with tc.tile_pool(name="p1", bufs=2) as p1, \
                     tc.tile_pool(name="p1s", bufs=2) as p1s, \
                     tc.tile_pool(name="gnps", bufs=1, space="PSUM") as gnps:
                    for ct in range(CT):
                        x_t = p1.tile([P, N], f32, name="xt")
                        nc.sync.dma_start(x_t[:], x_d[ct * P:(ct + 1) * P, :])
                        stats = p1s.tile([P, 8, 6], f32, name="stats")
                        for sg in range(8):
                            nc.vector.bn_stats(
                                stats[:, sg, :], x_t[:, sg * 512:(sg + 1) * 512])
                        mv = p1s.tile([P, 2], f32, name="mv")
                        nc.vector.bn_aggr(mv[:], stats[:])
                        # ms = [mean, E[x^2]] per channel
                        ms = p1s.tile([P, 2], f32, name="ms")
                        nc.vector.tensor_copy(ms[:, 0:1], mv[:, 0:1])
                        nc.vector.tensor_mul(ms[:, 1:2], mv[:, 0:1], mv[:, 0:1])
                        nc.vector.tensor_add(ms[:, 1:2], ms[:, 1:2], mv[:, 1:2])
                        # group aggregate: [8, 2] = (gmean, gE[x^2])
                        gps = gnps.tile([P // GS, 2], f32, name="gps")
                        nc.tensor.matmul(gps[:], lhsT=g16_sb[:], rhs=ms[:],
                                         start=True, stop=True)
                        gsb = p1s.tile([P // GS, 2], f32, name="gsb")
                        nc.vector.tensor_copy(gsb[:], gps[:])
                        gm2 = p1s.tile([P // GS, 1], f32, name="gm2")
                        nc.vector.tensor_mul(gm2[:], gsb[:, 0:1], gsb[:, 0:1])
                        nc.vector.tensor_sub(gsb[:, 1:2], gsb[:, 1:2], gm2[:])
                        # gsb[:,1] = 1/sqrt(gvar + eps)
                        nc.scalar.activation(gsb[:, 1:2], gsb[:, 1:2], AF.Sqrt,
                                             bias=eps_sb[:], scale=1.0)
                        nc.vector.reciprocal(gsb[:, 1:2], gsb[:, 1:2])
                        # broadcast group (mean, rstd) back to 128 channels
                        cps = gnps.tile([P, 2], f32, name="cps")
                        nc.tensor.matmul(cps[:], lhsT=gt_sb[:], rhs=gsb[:],
                                         start=True, stop=True)
                        scale_t = p1s.tile([P, 1], f32, name="scale")
                        nc.vector.tensor_mul(scale_t[:], cps[:, 1:2],
                                             gamma_sb[:, ct:ct + 1])
                        nbias_t = p1s.tile([P, 1], f32, name="nbias")
                        nc.vector.tensor_mul(nbias_t[:], cps[:, 0:1], scale_t[:])
                        nc.vector.tensor_sub(nbias_t[:], beta_sb[:, ct:ct + 1],
                                             nbias_t[:])
                        nc.scalar.activation(h_sb[:, ct, :], x_t[:], AF.Identity,
                                             bias=nbias_t[:], scale=scale_t[:])

                with tc.tile_pool(name="pjps", bufs=3, space="PSUM") as pjps:
                    for ct in range(CT):
                        for ib in range(NB):
                            qp = pjps.tile([P, FB], f32, name="mm")
                            for kt in range(CT):
                                nc.tensor.matmul(
                                    qp[:],
                                    lhsT=wq_sb[:, kt, ct * P:(ct + 1) * P],
                                    rhs=h_sb[:, kt, ib * FB:(ib + 1) * FB],
                                    start=(kt == 0), stop=(kt == CT - 1))
                            nc.scalar.activation(
                                q_sb[:, ct, ib * FB:(ib + 1) * FB], qp[:],
                                AF.Identity, bias=bq_sb[:, ct:ct + 1], scale=1.0)
                            kp = pjps.tile([P, FB], f32, name="mm")
                            for kt in range(CT):
                                nc.tensor.matmul(
                                    kp[:],
                                    lhsT=wk_sb[:, kt, ct * P:(ct + 1) * P],
                                    rhs=h_sb[:, kt, ib * FB:(ib + 1) * FB],
                                    start=(kt == 0), stop=(kt == CT - 1))
                            nc.scalar.activation(
                                k_sb[:, ct, ib * FB:(ib + 1) * FB], kp[:],
                                AF.Identity, bias=bk_sb[:, ct:ct + 1], scale=1.0)
                    for jb in range(JB):
                        vp = pjps.tile([P, C], f32, name="mm")
                        for kt in range(CT):
                            nc.tensor.matmul(
                                vp[:],
                                lhsT=h_sb[:, kt, jb * P:(jb + 1) * P],
                                rhs=wv_sb[:, kt, :],
                                start=(kt == 0), stop=(kt == CT - 1))
                        nc.scalar.activation(vt_sb[:, jb, :], vp[:], AF.Copy)

            # ---------------- phase 3+4: attention + output projection ------
            with tc.tile_pool(name="attnpool", bufs=1) as apool, \
                 tc.tile_pool(name="ptpool", bufs=PIPE + 3) as ptp, \
                 tc.tile_pool(name="sps", bufs=2, space="PSUM") as sps, \
                 tc.tile_pool(name="ops", bufs=1, space="PSUM") as ops, \
                 tc.tile_pool(name="dps", bufs=1, space="PSUM") as dps, \
                 tc.tile_pool(name="yps", bufs=1, space="PSUM") as yps, \
                 tc.tile_pool(name="mpool", bufs=2) as mp, \
                 tc.tile_pool(name="xrpool", bufs=3) as xrp, \
                 tc.tile_pool(name="outpool", bufs=3) as outp:
                attn_sb = apool.tile([P, CT, N], bf16, name="attn")

                def final_proj(ib):
                    for ct in range(CT):
                        yp = yps.tile([P, FB], f32, name="yp")
                        for kt in range(CT):
                            nc.tensor.matmul(
                                yp[:],
                                lhsT=wp_sb[:, kt, ct * P:(ct + 1) * P],
                                rhs=attn_sb[:, kt, ib * FB:(ib + 1) * FB],
                                start=(kt == 0), stop=(kt == CT - 1))
                        tb = outp.tile([P, FB], f32, name="tb")
                        nc.scalar.activation(tb[:], yp[:], AF.Identity,
                                             bias=bp_sb[:, ct:ct + 1], scale=1.0)
                        xr = xrp.tile([P, FB], f32, name="xr")
                        nc.sync.dma_start(
                            xr[:], x_d[ct * P:(ct + 1) * P, ib * FB:(ib + 1) * FB])
                        ot = outp.tile([P, FB], f32, name="ot")
                        nc.vector.tensor_add(ot[:], tb[:], xr[:])
                        nc.sync.dma_start(
                            out_d[ct * P:(ct + 1) * P, ib * FB:(ib + 1) * FB],
                            ot[:])

                for ib in range(NB):
                    o_tiles = [ops.tile([P, FB], f32, name=f"o{cs}")
                               for cs in range(CT)]
                    den = dps.tile([1, FB], f32, name="den")
                    pt_q = []

                    def consume(jb, pt):
                        nc.tensor.matmul(den[:], lhsT=ones_sb[:], rhs=pt[:],
                                         start=(jb == 0), stop=(jb == JB - 1))
                        for cs in range(CT):
                            nc.tensor.matmul(
                                o_tiles[cs][:],
                                lhsT=vt_sb[:, jb, cs * P:(cs + 1) * P],
                                rhs=pt[:],
                                start=(jb == 0), stop=(jb == JB - 1))

                    for jb in range(JB):
                        st = sps.tile([P, FB], f32, name="st")
                        for kt in range(CT):
                            nc.tensor.matmul(
                                st[:],
                                lhsT=k_sb[:, kt, jb * P:(jb + 1) * P],
                                rhs=q_sb[:, kt, ib * FB:(ib + 1) * FB],
                                start=(kt == 0), stop=(kt == CT - 1))
                        pt = ptp.tile([P, FB], bf16, name="pt")
                        nc.scalar.activation(pt[:], st[:], AF.Exp)
                        pt_q.append((jb, pt))
                        if jb == PIPE and ib > 0:
                            # overlap previous block's output projection with
                            # this block's score matmuls
                            final_proj(ib - 1)
                        if jb >= PIPE:
                            consume(*pt_q.pop(0))
                    while pt_q:
                        consume(*pt_q.pop(0))

                    rden = mp.tile([1, FB], f32, name="rden")
                    nc.vector.reciprocal(rden[:], den[:])
                    rdb = mp.tile([P, FB], f32, name="rdb")
                    nc.gpsimd.partition_broadcast(rdb[:], rden[:])
                    for cs in range(CT):
                        nc.vector.tensor_mul(
                            attn_sb[:, cs, ib * FB:(ib + 1) * FB],
                            o_tiles[cs][:], rdb[:])
                final_proj(NB - 1)

    nc.compile()
    return nc


def _host_inputs(x, gamma, beta, Wq, bq, Wk, bk, Wv, bv, Wp, bp):
    bf16 = ml_dtypes.bfloat16
    f32 = np.float32
    B = x.shape[0]
    s = 1.0 / math.sqrt(C)
    xs = np.asarray(x, f32).reshape(B, C, N)

    def fold(v):
        return np.asarray(v, f32).reshape(CT, P).T.copy()

    common = {
        "wqT": np.ascontiguousarray(np.asarray(Wq, f32).T * s).astype(bf16),
        "wkT": np.ascontiguousarray(np.asarray(Wk, f32).T).astype(bf16),
        "wvT": np.ascontiguousarray(np.asarray(Wv, f32).T).astype(bf16),
        "wpT": np.ascontiguousarray(np.asarray(Wp, f32).T).astype(bf16),
        "bq": fold(np.asarray(bq, f32) * s),
        "bk": fold(bk),
        "bp": fold(np.asarray(Wp, f32) @ np.asarray(bv, f32) + np.asarray(bp, f32)),
        "gamma": fold(gamma),
        "beta": fold(beta),
    }
    g16 = np.zeros((P, P // GS), f32)
    g16[np.arange(P), np.arange(P) // GS] = 1.0 / GS
    gt = np.zeros((P // GS, P), f32)
    gt[np.arange(P) // GS, np.arange(P)] = 1.0
    common["g16"] = g16
    common["gt"] = gt
    return [dict(common, x=np.ascontiguousarray(xs[b])) for b in range(B)]


def kernel(x, gamma, beta, Wq, bq, Wk, bk, Wv, bv, Wp, bp, _trace=False):
    from concourse.bass_utils import run_bass_kernel_spmd

    if "nc" not in _CACHE:
        _CACHE["nc"] = _build()
    nc = _CACHE["nc"]
    in_maps = _host_inputs(x, gamma, beta, Wq, bq, Wk, bk, Wv, bv, Wp, bp)
    B = len(in_maps)
    res = run_bass_kernel_spmd(nc, in_maps, core_ids=list(range(B)),
                               trace=_trace)
    out = np.stack([res.results[b]["out"] for b in range(B)])
    out = out.reshape(x.shape).astype(np.float32)
    if _trace:
        _CACHE["last_results"] = res
    return out
